# revision 24
# baseline (speedup 1.0000x reference)
"""ConvGRU Trainium2 kernel.

video [B=2, T=16, C=128, H=64, W=64] f32; 1x1-conv GRU over T.
Sharding: data-parallel over (B x H/4) -> 8 cores, each core owns
P = 16*64 = 1024 pixels for all T; weights replicated.

Layout per core: channels on partitions (128), pixels on the free dim.
Two pixel groups (G=2, PG=512) run as two independent recurrence
chains, SOFTWARE-PIPELINED half an iteration apart: each iteration
emits group A's full step t plus group B's deferred back-half of step
t-1 and B's front-half of step t.  This keeps every engine queue fed
with ready work (in-order engines never block one chain on the other).

PSUM (8 banks):
  r_ps [128,1024] halves = groups   (2 banks, single-buffered)
  z_ps [128,1024] halves = groups   (2 banks, single-buffered)
  c_ps [128,1024] halves = groups   (2 banks x 2 ping-pong)

Stage contents per group g, step t:
  front: WRH_g,WZH_g [PE] -> sig_r_g, zbar_g [ACT] -> rh_g,u_g [DVE],
         z_g=1-zbar [Pool]
  back : WHH_g [PE] -> tanh_g [ACT] -> v_g=z*c, h'_g=u+v [DVE],
         out-DMA [Pool queue]
  opens: x-side matmuls for t+1 (r/z after the gate reads free the
         banks; c into the ping-pong buffer)

t=0 is specialized (h=0).  x DMAs run two steps ahead on the SP queue.
Numerics: fp16 matmul inputs/gates/state, fp32 PSUM accum + fp32 bias.
"""

import os
import sys

import numpy as np

B, T, C, H, W = 2, 16, 128, 64, 64
NCORES = 8
HQ = H // 4          # 16 rows of H per core (4 H-slices x 2 batches = 8 cores)
P = HQ * W           # 1024 pixels per core
G = 2
PG = P // G          # 512 pixels per group

_PROG = None


def _ensure_paths():
    for p in ("/opt/trn_rl_repo",):
        if p not in sys.path and os.path.isdir(p):
            sys.path.append(p)


def _build():
    _ensure_paths()
    import concourse.bacc as bacc
    import concourse.tile as tile
    from concourse import mybir

    f32 = mybir.dt.float32
    f16 = mybir.dt.float16
    AF = mybir.ActivationFunctionType

    nc = bacc.Bacc(
        "TRN2", target_bir_lowering=False, debug=False, num_devices=NCORES
    )
    x_dram = nc.dram_tensor("x_seq", [T, C, P], f16, kind="ExternalInput")
    w_dram = nc.dram_tensor("wmats", [C, 8 * C], f16, kind="ExternalInput")
    b_dram = nc.dram_tensor("biases", [C, 4], f32, kind="ExternalInput")
    o_dram = nc.dram_tensor("out_seq", [T, C, P], f16, kind="ExternalOutput")

    x_ap = x_dram.ap()
    w_ap = w_dram.ap()
    b_ap = b_dram.ap()
    o_ap = o_dram.ap()

    WZX, WZH, WRX, WRH, WHX, WHH, BIR, BINZ = range(8)
    # bias columns: [br, -bz, bh, +bz]
    BR, NBZ, BH, PBZ = range(4)

    def gs(ap_, g):
        return ap_[:, g * PG : (g + 1) * PG]

    with tile.TileContext(nc) as tc:
        with (
            tc.tile_pool(name="consts", bufs=1) as consts,
            tc.tile_pool(name="xin", bufs=4) as xpool,
            tc.tile_pool(name="state", bufs=2) as spool,
            tc.tile_pool(name="work", bufs=2) as wk,
            tc.tile_pool(name="ps", bufs=1, space="PSUM") as ps,
        ):
            bt = consts.tile([C, 4], f32)
            nc.gpsimd.dma_start(bt[:], b_ap[:])
            wt = consts.tile([C, 8 * C], f16)
            nc.sync.dma_start(wt[:], w_ap[:])
            ones = consts.tile([C, PG], f16)
            nc.vector.memset(ones[:], 1.0)
            def wslice(i):
                return wt[:, i * C : (i + 1) * C]

            # rz_ps layout: [r_g0 | zneg_g0 | r_g1 | zneg_g1], one bank each.
            # Group g's sigmoid reads the contiguous [r_g | zneg_g] window;
            # biases (+br / -bz) are folded in by K=1 ones-matmuls and the
            # z-side weights are host-negated, so no bias AP is needed.
            rz_ps = ps.tile([C, 4 * PG], f32, tag="rz_ps", bufs=1)

            def rslice(g):
                return rz_ps[:, (2 * g) * PG : (2 * g + 1) * PG]

            def zslice(g):
                return rz_ps[:, (2 * g + 1) * PG : (2 * g + 2) * PG]

            def rzwin(g):
                return rz_ps[:, (2 * g) * PG : (2 * g + 2) * PG]

            def c_tile():
                return ps.tile([C, P], f32, tag="c_ps", bufs=2, name="c_ps")

            def gtile(tag):
                return wk.tile([C, PG], f16, tag=tag, name=tag)

            def htile(g):
                return spool.tile([C, PG], f16, tag=f"h16{g}", name=f"h16{g}")

            # -- preload the ACT table early with a tiny dummy sigmoid on
            #    SBUF data (no PSUM/warmup dependency) --
            wtmp = gtile("r16_0")
            nc.scalar.activation(
                wtmp[:, :4], bt[:, :4], AF.Sigmoid, bias=bt[:, BR : BR + 1]
            )
            c_cur = c_tile()

            def load_x(t):
                xt = xpool.tile([C, P], f16, tag="x", name="x")
                nc.sync.dma_start(xt[:], x_ap[t])
                return xt

            # ---- pipeline stage emitters ----
            def front(g, t, h_prev):
                """r/z pre-act close + gate sigmoids + rh/u/z.
                Returns ctx needed by back()."""
                nc.tensor.matmul(rslice(g), wslice(WRH), h_prev[:],
                                 start=False, stop=True)
                nc.tensor.matmul(zslice(g), wslice(WZH), h_prev[:],
                                 start=False, stop=True)
                rzb = wk.tile([C, 2 * PG], f16, tag=f"rzb_{g}",
                              name=f"rzb_{g}")
                nc.scalar.activation(rzb[:], rzwin(g), AF.Sigmoid)
                r16 = rzb[:, :PG]
                zb16 = rzb[:, PG:]
                rh16 = gtile(f"rh16_{g}")
                u16 = gtile(f"u16_{g}")
                z16 = gtile(f"z16_{g}")
                nc.vector.tensor_mul(rh16[:], r16, h_prev[:])
                nc.vector.tensor_mul(u16[:], zb16, h_prev[:])
                nc.gpsimd.tensor_scalar(z16[:], zb16, -1.0, 1.0,
                                        mybir.AluOpType.mult,
                                        mybir.AluOpType.add)
                return {"rh": rh16, "u": u16, "z": z16, "t": t}

            def back_pe(g, ctx, c_ps_t):
                nc.tensor.matmul(gs(c_ps_t, g), wslice(WHH), ctx["rh"][:],
                                 start=False, stop=True)

            def back_rest(g, ctx, c_ps_t):
                t = ctx["t"]
                c16 = gtile(f"c16_{g}")
                nc.scalar.activation(c16[:], gs(c_ps_t, g), AF.Tanh,
                                     bias=bt[:, BH : BH + 1])
                v16 = gtile(f"v16_{g}")
                h_new = htile(g)
                nc.vector.tensor_mul(v16[:], ctx["z"][:], c16[:])
                nc.vector.tensor_add(h_new[:], ctx["u"][:], v16[:])
                if (t + g) % 2:
                    nc.gpsimd.dma_start(o_ap[t, :, g * PG : (g + 1) * PG],
                                        h_new[:])
                else:
                    nc.sync.dma_start(o_ap[t, :, g * PG : (g + 1) * PG],
                                      h_new[:])
                return h_new

            def open_c(g, xt, c_ps_new):
                nc.tensor.matmul(gs(c_ps_new, g), wslice(WHX), gs(xt, g),
                                 start=True, stop=False)

            def open_rz(g, xt):
                # bias broadcast (row-0 bias matrix @ ones) then x part
                nc.tensor.matmul(rslice(g), wslice(BIR), ones[:],
                                 start=True, stop=False)
                nc.tensor.matmul(rslice(g), wslice(WRX), gs(xt, g),
                                 start=False, stop=False)
                nc.tensor.matmul(zslice(g), wslice(BINZ), ones[:],
                                 start=True, stop=False)
                nc.tensor.matmul(zslice(g), wslice(WZX), gs(xt, g),
                                 start=False, stop=False)

            # ---------------- t = 0 (h = 0) ----------------
            xs = {0: load_x(0), 1: load_x(1)}
            x0 = xs[0]
            for g in range(G):
                nc.tensor.matmul(zslice(g), wslice(WZX), gs(x0, g),
                                 start=True, stop=True)
                nc.tensor.matmul(gs(c_cur, g), wslice(WHX), gs(x0, g),
                                 start=True, stop=True)
            xs[2] = load_x(2)
            h_a = None
            h_b = None
            for g in range(G):
                z16 = gtile(f"z16_{g}")
                c16 = gtile(f"c16_{g}")
                nc.scalar.activation(z16[:], zslice(g), AF.Sigmoid,
                                     bias=bt[:, PBZ : PBZ + 1], scale=-1.0)
                nc.scalar.activation(c16[:], gs(c_cur, g), AF.Tanh,
                                     bias=bt[:, BH : BH + 1])
                hg = htile(g)
                nc.vector.tensor_mul(hg[:], z16[:], c16[:])
                nc.sync.dma_start(o_ap[0, :, g * PG : (g + 1) * PG], hg[:])
                if g == 0:
                    h_a = hg
                else:
                    h_b = hg

            # opens for step 1 (both groups; banks are free)
            c_cur = c_tile()
            for g in range(G):
                open_c(g, xs[1], c_cur)
                open_rz(g, xs[1])

            # ---------------- pipeline prologue (iter 1) ----------------
            # A full step 1; B front of step 1; opens for 2.
            actx = front(0, 1, h_a)
            back_pe(0, actx, c_cur)
            back_rest_h = back_rest(0, actx, c_cur)
            h_a = back_rest_h
            bctx = front(1, 1, h_b)
            c_next = c_tile()
            xs[3] = load_x(3)
            open_c(0, xs[2], c_next)
            open_c(1, xs[2], c_next)
            open_rz(0, xs[2])
            open_rz(1, xs[2])
            c_prev, c_cur = c_cur, c_next

            # ---------------- steady iterations tau = 2..T-1 ----------------
            for t in range(2, T):
                opens = t + 1 < T
                x_n = xs[t + 1] if opens else None
                if t + 2 < T:
                    xs[t + 2] = load_x(t + 2)

                # A front (step t)
                new_actx = front(0, t, h_a)
                # B back (step t-1) -- deps all ready
                back_pe(1, bctx, c_prev)
                # A back (step t)
                back_pe(0, new_actx, c_cur)
                h_b = back_rest(1, bctx, c_prev)
                h_a = back_rest(0, new_actx, c_cur)
                # B front (step t)
                new_bctx = front(1, t, h_b)

                # opens for t+1
                if opens:
                    c_next = c_tile()
                    open_c(0, x_n, c_next)
                    open_rz(0, x_n)
                    open_c(1, x_n, c_next)
                    open_rz(1, x_n)
                    c_prev, c_cur = c_cur, c_next
                else:
                    c_prev = c_cur

                actx, bctx = new_actx, new_bctx

            # ---------------- epilogue: B back of step T-1 ----------------
            back_pe(1, bctx, c_prev)
            back_rest(1, bctx, c_prev)

    nc.compile()
    return nc


def _get_prog():
    global _PROG
    if _PROG is None:
        _PROG = _build()
    return _PROG


def _make_in_maps(video, Wz, bz, Wr, br, Wh, bh):
    bmat_r = np.zeros((C, C), np.float32)
    bmat_r[0, :] = br
    bmat_nz = np.zeros((C, C), np.float32)
    bmat_nz[0, :] = -bz
    w6 = np.concatenate(
        [
            -Wz[:, :C].T, -Wz[:, C:].T,
            Wr[:, :C].T, Wr[:, C:].T,
            Wh[:, :C].T, Wh[:, C:].T,
            bmat_r, bmat_nz,
        ],
        axis=1,
    ).astype(np.float16)
    b4 = np.stack([br, -bz, bh, bz], axis=1).astype(np.float32)
    in_maps = []
    for core in range(NCORES):
        b_, q = divmod(core, 4)
        xsl = np.ascontiguousarray(
            video[b_, :, :, q * HQ : (q + 1) * HQ, :]
        ).reshape(T, C, P).astype(np.float16)
        in_maps.append({"x_seq": xsl, "wmats": w6, "biases": b4})
    return in_maps


def kernel(video, Wz, bz, Wr, br, Wh, bh):
    _ensure_paths()
    from concourse.bass_utils import run_bass_kernel_spmd

    video = np.asarray(video, dtype=np.float32)
    nc = _get_prog()
    in_maps = _make_in_maps(video, Wz, bz, Wr, br, Wh, bh)
    res = run_bass_kernel_spmd(nc, in_maps, list(range(NCORES)))

    out = np.empty((B, T, C, H, W), np.float32)
    for core in range(NCORES):
        b_, q = divmod(core, 4)
        out[b_, :, :, q * HQ : (q + 1) * HQ, :] = np.asarray(
            res.results[core]["out_seq"]
        ).astype(np.float32).reshape(T, C, HQ, W)
    return out


# revision 25
# speedup vs baseline: 1.4617x; 1.4617x over previous
"""ConvGRU Trainium2 kernel.

video [B=2, T=16, C=128, H=64, W=64] f32; 1x1-conv GRU over T.
Sharding: data-parallel over (B x H/4) -> 8 cores, each core owns
P = 16*64 = 1024 pixels for all T; weights replicated.

Layout per core: channels on partitions (128), pixels on the free dim.
Two pixel groups (G=2, PG=512) run as two independent recurrence
chains, SOFTWARE-PIPELINED half an iteration apart: each iteration
emits group A's full step t plus group B's deferred back-half of step
t-1 and B's front-half of step t.  This keeps every engine queue fed
with ready work (in-order engines never block one chain on the other).

PSUM (8 banks):
  r_ps [128,1024] halves = groups   (2 banks, single-buffered)
  z_ps [128,1024] halves = groups   (2 banks, single-buffered)
  c_ps [128,1024] halves = groups   (2 banks x 2 ping-pong)

Stage contents per group g, step t:
  front: WRH_g,WZH_g [PE] -> sig_r_g, zbar_g [ACT] -> rh_g,u_g [DVE],
         z_g=1-zbar [Pool]
  back : WHH_g [PE] -> tanh_g [ACT] -> v_g=z*c, h'_g=u+v [DVE],
         out-DMA [Pool queue]
  opens: x-side matmuls for t+1 (r/z after the gate reads free the
         banks; c into the ping-pong buffer)

t=0 is specialized (h=0).  x DMAs run two steps ahead on the SP queue.
Numerics: fp16 matmul inputs/gates/state, fp32 PSUM accum + fp32 bias.
"""

import os
import sys

import numpy as np

B, T, C, H, W = 2, 16, 128, 64, 64
NCORES = 8
HQ = H // 4          # 16 rows of H per core (4 H-slices x 2 batches = 8 cores)
P = HQ * W           # 1024 pixels per core
G = 2
PG = P // G          # 512 pixels per group

_PROG = None


def _ensure_paths():
    for p in ("/opt/trn_rl_repo",):
        if p not in sys.path and os.path.isdir(p):
            sys.path.append(p)


def _build():
    _ensure_paths()
    import concourse.bacc as bacc
    import concourse.tile as tile
    from concourse import mybir

    f32 = mybir.dt.float32
    f16 = mybir.dt.float16
    AF = mybir.ActivationFunctionType

    nc = bacc.Bacc(
        "TRN2", target_bir_lowering=False, debug=False, num_devices=NCORES
    )
    x_dram = nc.dram_tensor("x_seq", [T, C, P], f16, kind="ExternalInput")
    w_dram = nc.dram_tensor("wmats", [C, 6 * C], f16, kind="ExternalInput")
    b_dram = nc.dram_tensor("biases", [C, 4], f32, kind="ExternalInput")
    o_dram = nc.dram_tensor("out_seq", [T, C, P], f16, kind="ExternalOutput")

    x_ap = x_dram.ap()
    w_ap = w_dram.ap()
    b_ap = b_dram.ap()
    o_ap = o_dram.ap()

    WZX, WZH, WRX, WRH, WHX, WHH = range(6)
    # bias columns: [br, -bz, bh, +bz]
    BR, NBZ, BH, PBZ = range(4)

    def gs(ap_, g):
        return ap_[:, g * PG : (g + 1) * PG]

    with tile.TileContext(nc) as tc:
        with (
            tc.tile_pool(name="consts", bufs=1) as consts,
            tc.tile_pool(name="xin", bufs=4) as xpool,
            tc.tile_pool(name="state", bufs=2) as spool,
            tc.tile_pool(name="work", bufs=2) as wk,
            tc.tile_pool(name="ps", bufs=1, space="PSUM") as ps,
        ):
            bt = consts.tile([C, 4], f32)
            nc.gpsimd.dma_start(bt[:], b_ap[:])
            wt = consts.tile([C, 6 * C], f16)
            nc.sync.dma_start(wt[:], w_ap[:])
            def wslice(i):
                return wt[:, i * C : (i + 1) * C]

            # rz_ps layout: [r_g0 | zneg_g0 | r_g1 | zneg_g1], one bank each.
            # Group g's sigmoid reads the contiguous [r_g | zneg_g] window;
            # biases (+br / -bz) are folded in by K=1 ones-matmuls and the
            # z-side weights are host-negated, so no bias AP is needed.
            rz_ps = ps.tile([C, 4 * PG], f32, tag="rz_ps", bufs=1)

            def rslice(g):
                return rz_ps[:, (2 * g) * PG : (2 * g + 1) * PG]

            def zslice(g):
                return rz_ps[:, (2 * g + 1) * PG : (2 * g + 2) * PG]

            def rzwin(g):
                return rz_ps[:, (2 * g) * PG : (2 * g + 2) * PG]

            def c_tile():
                return ps.tile([C, P], f32, tag="c_ps", bufs=2, name="c_ps")

            def gtile(tag):
                return wk.tile([C, PG], f16, tag=tag, name=tag)

            def htile(g):
                return spool.tile([C, PG], f16, tag=f"h16{g}", name=f"h16{g}")

            # -- preload the ACT table early with a tiny dummy sigmoid on
            #    SBUF data (no PSUM/warmup dependency) --
            wtmp = gtile("r16_0")
            nc.scalar.activation(
                wtmp[:, :4], bt[:, :4], AF.Sigmoid, bias=bt[:, BR : BR + 1]
            )
            c_cur = c_tile()

            def load_x(t):
                xt = xpool.tile([C, P], f16, tag="x", name="x")
                nc.sync.dma_start(xt[:], x_ap[t])
                return xt

            # ---- pipeline stage emitters ----
            def front(g, t, h_prev):
                """r/z pre-act close + gate sigmoids + rh/u/z.
                Returns ctx needed by back()."""
                nc.tensor.matmul(rslice(g), wslice(WRH), h_prev[:],
                                 start=False, stop=True)
                nc.tensor.matmul(zslice(g), wslice(WZH), h_prev[:],
                                 start=False, stop=True)
                r16 = gtile(f"r16_{g}")
                zb16 = gtile(f"zb16_{g}")
                nc.scalar.activation(r16[:], rslice(g), AF.Sigmoid,
                                     bias=bt[:, BR : BR + 1])
                nc.scalar.activation(zb16[:], zslice(g), AF.Sigmoid,
                                     bias=bt[:, NBZ : NBZ + 1])
                rh16 = gtile(f"rh16_{g}")
                u16 = gtile(f"u16_{g}")
                z16 = gtile(f"z16_{g}")
                nc.vector.tensor_mul(rh16[:], r16[:], h_prev[:])
                nc.vector.tensor_mul(u16[:], zb16[:], h_prev[:])
                nc.vector.tensor_scalar(z16[:], zb16[:], -1.0, 1.0,
                                        mybir.AluOpType.mult,
                                        mybir.AluOpType.add)
                return {"rh": rh16, "u": u16, "z": z16, "t": t}

            def back_pe(g, ctx, c_ps_t):
                nc.tensor.matmul(gs(c_ps_t, g), wslice(WHH), ctx["rh"][:],
                                 start=False, stop=True)

            def back_rest(g, ctx, c_ps_t):
                t = ctx["t"]
                c16 = gtile(f"c16_{g}")
                nc.scalar.activation(c16[:], gs(c_ps_t, g), AF.Tanh,
                                     bias=bt[:, BH : BH + 1])
                v16 = gtile(f"v16_{g}")
                h_new = htile(g)
                nc.vector.tensor_mul(v16[:], ctx["z"][:], c16[:])
                nc.vector.tensor_add(h_new[:], ctx["u"][:], v16[:])
                if (t + g) % 2:
                    nc.gpsimd.dma_start(o_ap[t, :, g * PG : (g + 1) * PG],
                                        h_new[:])
                else:
                    nc.sync.dma_start(o_ap[t, :, g * PG : (g + 1) * PG],
                                      h_new[:])
                return h_new

            def open_c(g, xt, c_ps_new):
                nc.tensor.matmul(gs(c_ps_new, g), wslice(WHX), gs(xt, g),
                                 start=True, stop=False)

            def open_rz(g, xt):
                nc.tensor.matmul(rslice(g), wslice(WRX), gs(xt, g),
                                 start=True, stop=False)
                nc.tensor.matmul(zslice(g), wslice(WZX), gs(xt, g),
                                 start=True, stop=False)

            # ---------------- t = 0 (h = 0) ----------------
            xs = {0: load_x(0), 1: load_x(1)}
            x0 = xs[0]
            for g in range(G):
                nc.tensor.matmul(zslice(g), wslice(WZX), gs(x0, g),
                                 start=True, stop=True)
                nc.tensor.matmul(gs(c_cur, g), wslice(WHX), gs(x0, g),
                                 start=True, stop=True)
            xs[2] = load_x(2)
            h_a = None
            h_b = None
            for g in range(G):
                z16 = gtile(f"z16_{g}")
                c16 = gtile(f"c16_{g}")
                nc.scalar.activation(z16[:], zslice(g), AF.Sigmoid,
                                     bias=bt[:, PBZ : PBZ + 1], scale=-1.0)
                nc.scalar.activation(c16[:], gs(c_cur, g), AF.Tanh,
                                     bias=bt[:, BH : BH + 1])
                hg = htile(g)
                nc.vector.tensor_mul(hg[:], z16[:], c16[:])
                nc.sync.dma_start(o_ap[0, :, g * PG : (g + 1) * PG], hg[:])
                if g == 0:
                    h_a = hg
                else:
                    h_b = hg

            # opens for step 1 (both groups; banks are free)
            c_cur = c_tile()
            for g in range(G):
                open_c(g, xs[1], c_cur)
                open_rz(g, xs[1])

            # ---------------- pipeline prologue (iter 1) ----------------
            # A full step 1; B front of step 1; opens for 2.
            actx = front(0, 1, h_a)
            back_pe(0, actx, c_cur)
            back_rest_h = back_rest(0, actx, c_cur)
            h_a = back_rest_h
            bctx = front(1, 1, h_b)
            c_next = c_tile()
            xs[3] = load_x(3)
            open_c(0, xs[2], c_next)
            open_c(1, xs[2], c_next)
            open_rz(0, xs[2])
            open_rz(1, xs[2])
            c_prev, c_cur = c_cur, c_next

            # ---------------- steady iterations tau = 2..T-1 ----------------
            for t in range(2, T):
                opens = t + 1 < T
                x_n = xs[t + 1] if opens else None
                if t + 2 < T:
                    xs[t + 2] = load_x(t + 2)

                # A front (step t)
                new_actx = front(0, t, h_a)
                # B back (step t-1) -- deps all ready
                back_pe(1, bctx, c_prev)
                # A back (step t)
                back_pe(0, new_actx, c_cur)
                h_b = back_rest(1, bctx, c_prev)
                h_a = back_rest(0, new_actx, c_cur)
                # B front (step t)
                new_bctx = front(1, t, h_b)

                # opens for t+1
                if opens:
                    c_next = c_tile()
                    open_c(0, x_n, c_next)
                    open_rz(0, x_n)
                    open_c(1, x_n, c_next)
                    open_rz(1, x_n)
                    c_prev, c_cur = c_cur, c_next
                else:
                    c_prev = c_cur

                actx, bctx = new_actx, new_bctx

            # ---------------- epilogue: B back of step T-1 ----------------
            back_pe(1, bctx, c_prev)
            back_rest(1, bctx, c_prev)

    nc.compile()
    return nc


def _get_prog():
    global _PROG
    if _PROG is None:
        _PROG = _build()
    return _PROG


def _make_in_maps(video, Wz, bz, Wr, br, Wh, bh):
    w6 = np.concatenate(
        [
            -Wz[:, :C].T, -Wz[:, C:].T,
            Wr[:, :C].T, Wr[:, C:].T,
            Wh[:, :C].T, Wh[:, C:].T,
        ],
        axis=1,
    ).astype(np.float16)
    b4 = np.stack([br, -bz, bh, bz], axis=1).astype(np.float32)
    in_maps = []
    for core in range(NCORES):
        b_, q = divmod(core, 4)
        xsl = np.ascontiguousarray(
            video[b_, :, :, q * HQ : (q + 1) * HQ, :]
        ).reshape(T, C, P).astype(np.float16)
        in_maps.append({"x_seq": xsl, "wmats": w6, "biases": b4})
    return in_maps


def kernel(video, Wz, bz, Wr, br, Wh, bh):
    _ensure_paths()
    from concourse.bass_utils import run_bass_kernel_spmd

    video = np.asarray(video, dtype=np.float32)
    nc = _get_prog()
    in_maps = _make_in_maps(video, Wz, bz, Wr, br, Wh, bh)
    res = run_bass_kernel_spmd(nc, in_maps, list(range(NCORES)))

    out = np.empty((B, T, C, H, W), np.float32)
    for core in range(NCORES):
        b_, q = divmod(core, 4)
        out[b_, :, :, q * HQ : (q + 1) * HQ, :] = np.asarray(
            res.results[core]["out_seq"]
        ).astype(np.float32).reshape(T, C, HQ, W)
    return out
